# revision 22
# baseline (speedup 1.0000x reference)
"""Multi-head attention TRN2 Bass kernel (B=4, S=2048, E=2048, H=16, D=128).

Sharding: 2 heads per core (tensor parallel over H=16 across 8 cores).
Each core computes q/k/v projections for its 2 heads over all batches,
attention, and a partial out-projection (its heads' columns of W_out).
Host sums the 8 fp32 partial outputs (the "all-reduce") and transposes.

v2 schedule: software-pipelined across batches. The QKV projection of
batch b+1 is emitted interleaved with the attention of batch b so the
in-order PE queue always has dense matmul work while the scalar engine
runs exp (attention alone is exp-bound: ~1.2us scalar vs 0.85us PE per
key-chunk). Scores run one key-chunk ahead of the attn*V matmuls, the
out-projection of query-block 0 interleaves with attention of block 1,
and the softmax denominator is a gpsimd partition-reduce instead of a
PE ones-matmul. PSUM: pq(2) + sc(4) + oc(2) = 8 banks.

Device layouts (per core):
  xt   [B, E, S]  bf16   x transposed per batch (feature-major)
  wqk  [E, 4D]    bf16   W_q/W_k columns for heads (q0|q1|k0|k1)
  wv   [E, 2D]    bf16   W_v columns (v0|v1)
  wo   [2D, E]    bf16   W_out^T rows for this core's head channels
  yt   [B, E, S]  f16    partial output, feature-major (ExternalOutput)
"""

import os
import sys

sys.path.insert(0, "/opt/trn_rl_repo")

import numpy as np
import ml_dtypes

B, S, E = 4, 2048, 2048
H, D = 16, 128
NCORES = 8
HPC = H // NCORES  # 2 heads per core
SCALE = 1.0 / float(np.sqrt(D))

EC = E // 128     # 16 contraction chunks
TCX = 1024        # x sbuf tile width (tokens)
KC = S // 128     # 16 key chunks
NQP = S // 1024   # 2 query chunk-pairs


def _interleave(streams):
    """streams: generators that emit instructions inside next() and yield
    the unit's PE cost. Alternate emission so cumulative costs stay even."""
    spent = [0.0] * len(streams)
    alive = [True] * len(streams)
    while any(alive):
        j = min((i for i in range(len(streams)) if alive[i]), key=lambda i: spent[i])
        try:
            spent[j] += next(streams[j])
        except StopIteration:
            alive[j] = False


def _build():
    import concourse.bass as bass
    import concourse.tile as tile
    from concourse import bacc, mybir

    bf = mybir.dt.bfloat16
    f16 = mybir.dt.float16
    f32 = mybir.dt.float32
    ADD = mybir.AluOpType.add
    MULT = mybir.AluOpType.mult
    EXP = mybir.ActivationFunctionType.Exp

    nc = bacc.Bacc(
        "TRN2", target_bir_lowering=False, debug=False, num_devices=NCORES
    )
    xt = nc.dram_tensor("xt", [B, E, S], bf, kind="ExternalInput").ap()
    wqk = nc.dram_tensor("wqk", [E, 4 * D], bf, kind="ExternalInput").ap()
    wv = nc.dram_tensor("wv", [E, 2 * D], bf, kind="ExternalInput").ap()
    wo = nc.dram_tensor("wo", [2 * D, E], bf, kind="ExternalInput").ap()
    yt = nc.dram_tensor("yt", [B, E, S], f16, kind="ExternalOutput").ap()

    with tile.TileContext(nc) as tc:
        with (
            tc.tile_pool(name="wp", bufs=1) as wp,
            tc.tile_pool(name="xp", bufs=20) as xp,
            tc.tile_pool(name="qkp", bufs=8) as qkp,
            tc.tile_pool(name="vp", bufs=32) as vp,
            tc.tile_pool(name="op", bufs=8) as osp,
            tc.tile_pool(name="ptp", bufs=6) as ptp,
            tc.tile_pool(name="accp", bufs=3) as accp,
            tc.tile_pool(name="dbp", bufs=2) as dbp,
            tc.tile_pool(name="yp", bufs=4) as yp,
            tc.tile_pool(name="oup", bufs=6) as oup,
            tc.tile_pool(name="psp", bufs=2, space="PSUM") as psp,
        ):
            wqk_t = [
                wp.tile([128, 4 * D], bf, tag=f"wqk{ec}", name=f"wqk_{ec}")
                for ec in range(EC)
            ]
            wv_t = [
                wp.tile([128, 2 * D], bf, tag=f"wv{ec}", name=f"wv_{ec}")
                for ec in range(EC)
            ]
            wo_t = [
                wp.tile([128, E], bf, tag=f"wo{cc}", name=f"wo_{cc}")
                for cc in range(HPC)
            ]
            ones_sb = wp.tile([128, 1], f16, tag="ones", name="ones_sb")
            nc.vector.memset(ones_sb, 1.0)

            # per-batch state, keyed b%2 for double buffering
            qk_tiles = {}
            v_tiles = {}
            out_sb = {}

            def load_wv():
                for ec in range(EC):
                    nc.scalar.dma_start(wv_t[ec], wv[ec * 128 : (ec + 1) * 128, :])

            def load_wo():
                for cc in range(HPC):
                    nc.scalar.dma_start(wo_t[cc], wo[cc * 128 : (cc + 1) * 128, :])

            def qkv_stream(b):
                """QKV projection for batch b: x DMAs + 16 QK chains + 16 V
                chains. Emits units, yields their PE cost (us)."""
                x_t = {}

                def dma_x(tcx):
                    tiles = []
                    for ec in range(EC):
                        xtile = xp.tile(
                            [128, TCX], bf, tag="x", name=f"x_{b}_{tcx}_{ec}"
                        )
                        nc.sync.dma_start(
                            xtile,
                            xt[
                                b,
                                ec * 128 : (ec + 1) * 128,
                                tcx * TCX : (tcx + 1) * TCX,
                            ],
                        )
                        tiles.append(xtile)
                    x_t[tcx] = tiles

                if b == 0:
                    # weights on the scalar DGE ring, x on the sync ring, so
                    # the first chain's operands land in parallel
                    x_t[0] = []
                    for ec in range(EC):
                        nc.scalar.dma_start(
                            wqk_t[ec], wqk[ec * 128 : (ec + 1) * 128, :]
                        )
                        xtile = xp.tile([128, TCX], bf, tag="x", name=f"x_0_0_{ec}")
                        nc.sync.dma_start(
                            xtile, xt[0, ec * 128 : (ec + 1) * 128, 0:TCX]
                        )
                        x_t[0].append(xtile)
                else:
                    dma_x(0)
                yield 0.1

                qk_tiles[b % 2] = [
                    qkp.tile([128, S], bf, tag="qk", name=f"qk_{b}_{mc}")
                    for mc in range(4)
                ]
                v_tiles[b % 2] = [
                    vp.tile([128, 2 * D], f16, tag="v", name=f"v_{b}_{kc}")
                    for kc in range(KC)
                ]

                def v_chain(tcx, tsub):
                    pv = psp.tile(
                        [128, 2 * D], f32, tag="pq", name=f"pv_{b}_{tcx}_{tsub}"
                    )
                    for ec in range(EC):
                        nc.tensor.matmul(
                            pv,
                            lhsT=x_t[tcx][ec][:, tsub * 128 : (tsub + 1) * 128],
                            rhs=wv_t[ec],
                            start=(ec == 0),
                            stop=(ec == EC - 1),
                        )
                    kc = tcx * (TCX // 128) + tsub
                    if b == 0:
                        nc.vector.tensor_copy(v_tiles[b % 2][kc], pv)
                    else:
                        nc.scalar.copy(v_tiles[b % 2][kc], pv)

                nchain = 0
                for tcx in range(S // TCX):
                    # QK chains: out [D, 512] per (mc, tcs)
                    for mc in range(4):
                        for tcs in range(2):
                            pq = psp.tile(
                                [128, 512], f32, tag="pq",
                                name=f"pq_{b}_{tcx}_{mc}_{tcs}",
                            )
                            for ec in range(EC):
                                nc.tensor.matmul(
                                    pq,
                                    lhsT=wqk_t[ec][:, mc * 128 : (mc + 1) * 128],
                                    rhs=x_t[tcx][ec][:, tcs * 512 : (tcs + 1) * 512],
                                    start=(ec == 0),
                                    stop=(ec == EC - 1),
                                )
                            tchunk = tcx * TCX + tcs * 512
                            drain = nc.vector.tensor_copy if b == 0 else nc.scalar.copy
                            drain(
                                qk_tiles[b % 2][mc][:, tchunk : tchunk + 512], pq
                            )
                            nchain += 1
                            if b == 0 and tcx == 0:
                                if nchain == 1:
                                    load_wv()
                                elif nchain == 3:
                                    load_wo()
                            if tcx == 0 and nchain == 2:
                                dma_x(1)
                            yield 3.41
                    # V chains: out [128 tokens, 2D]
                    for tsub in range(TCX // 128):
                        if b == B - 1 and tcx == 1:
                            # defer: emitted inside attn_stream(B-1) as PE
                            # filler (attention alone is exp-bound)
                            deferred_v.append(
                                (1.71, (lambda t, s: lambda: v_chain(t, s))(tcx, tsub))
                            )
                        else:
                            v_chain(tcx, tsub)
                            yield 1.71

            def attn_chunk_units(b, qp, h):
                """Attention for one (query-block, head): scores run one key
                chunk ahead of attn*V. Returns a list of (cost, fn) units."""
                q_t = qk_tiles[b % 2][h]
                k_t = qk_tiles[b % 2][2 + h]
                vt = v_tiles[b % 2]
                q0 = qp * 1024
                state = {}

                def scores(kc):
                    sps = psp.tile(
                        [128, 1024], f32, tag="sc", name=f"s_{b}_{h}_{qp}_{kc}"
                    )
                    for half in range(2):
                        nc.tensor.matmul(
                            sps[:, half * 512 : (half + 1) * 512],
                            lhsT=k_t[:, kc * 128 : (kc + 1) * 128],
                            rhs=q_t[:, q0 + half * 512 : q0 + (half + 1) * 512],
                            start=True,
                            stop=True,
                        )
                    pt = ptp.tile(
                        [128, 1024], f16, tag="pt", name=f"pt_{b}_{h}_{qp}_{kc}"
                    )
                    nc.scalar.activation(pt, sps, EXP, scale=SCALE)
                    state[kc] = pt

                def attnv(kc):
                    pt = state[kc]
                    for half, ps in ((0, state["oa"]), (1, state["ob"])):
                        nc.tensor.matmul(
                            ps,
                            lhsT=vt[kc][:, h * 128 : (h + 1) * 128],
                            rhs=pt[:, half * 512 : (half + 1) * 512],
                            start=(kc == 0),
                            stop=(kc == KC - 1),
                        )
                    acc = state["acc"]
                    if kc == 1:
                        nc.vector.tensor_tensor(acc, state[0], pt, ADD)
                    elif kc > 1:
                        nc.vector.tensor_tensor(acc, acc, pt, ADD)
                    if kc >= 1:
                        del state[kc - 1]

                def prologue():
                    state["oa"] = psp.tile(
                        [128, 512], f32, tag="oc", name=f"oa_{b}_{h}_{qp}"
                    )
                    state["ob"] = psp.tile(
                        [128, 512], f32, tag="oc", name=f"ob_{b}_{h}_{qp}"
                    )
                    state["acc"] = accp.tile(
                        [128, 1024], f16, tag="acc", name=f"acc_{b}_{h}_{qp}"
                    )
                    scores(0)

                def mid(kc):
                    def fn():
                        scores(kc)
                        attnv(kc - 1)

                    return fn

                def epilogue():
                    attnv(KC - 1)
                    ou_a = oup.tile(
                        [128, 512], f16, tag="ou", name=f"oua_{b}_{h}_{qp}"
                    )
                    ou_b = oup.tile(
                        [128, 512], f16, tag="ou", name=f"oub_{b}_{h}_{qp}"
                    )
                    nc.vector.tensor_copy(ou_a, state["oa"])
                    nc.vector.tensor_copy(ou_b, state["ob"])
                    rec = dbp.tile([1, 1024], f32, tag="rec", name=f"rec_{b}_{h}_{qp}")
                    for dh in range(2):
                        den_ps = psp.tile(
                            [1, 512], f32, tag="pq", name=f"den_{b}_{h}_{qp}_{dh}"
                        )
                        nc.tensor.matmul(
                            den_ps,
                            lhsT=ones_sb,
                            rhs=state["acc"][:, dh * 512 : (dh + 1) * 512],
                            start=True,
                            stop=True,
                        )
                        nc.vector.reciprocal_approx_fast(
                            out=rec[:, dh * 512 : (dh + 1) * 512], in_=den_ps
                        )
                    dbc = dbp.tile(
                        [128, 1024], f32, tag="dbc", name=f"dbc_{b}_{h}_{qp}"
                    )
                    nc.gpsimd.partition_broadcast(dbc, rec)
                    ot = out_sb[(b % 2, qp, h)]
                    nc.vector.tensor_tensor(ot[:, :512], ou_a, dbc[:, :512], MULT)
                    nc.vector.tensor_tensor(ot[:, 512:], ou_b, dbc[:, 512:], MULT)

                units = [(0.85, prologue)]
                units += [(0.85, mid(kc)) for kc in range(1, KC)]
                units.append((1.0, epilogue))
                return units

            def outproj_units(b, qp):
                units = []

                def fc_unit(fc):
                    def fn():
                        yps = psp.tile(
                            [128, 1024], f32, tag="sc", name=f"y_{b}_{qp}_{fc}"
                        )
                        for half in range(2):
                            for cc in range(HPC):
                                nc.tensor.matmul(
                                    yps[:, half * 512 : (half + 1) * 512],
                                    lhsT=wo_t[cc][:, fc * 128 : (fc + 1) * 128],
                                    rhs=out_sb[(b % 2, qp, cc)][
                                        :, half * 512 : (half + 1) * 512
                                    ],
                                    start=(cc == 0),
                                    stop=(cc == HPC - 1),
                                )
                        y_sb = yp.tile(
                            [128, 1024], f16, tag="y", name=f"ysb_{b}_{qp}_{fc}"
                        )
                        nc.vector.tensor_copy(y_sb, yps)
                        nc.gpsimd.dma_start(
                            yt[
                                b,
                                fc * 128 : (fc + 1) * 128,
                                qp * 1024 : (qp + 1) * 1024,
                            ],
                            y_sb,
                        )

                    return fn

                for fc in range(E // 128):
                    units.append((0.85, fc_unit(fc)))
                return units

            def attn_stream(b):
                for qp in range(NQP):
                    for h in range(HPC):
                        out_sb[(b % 2, qp, h)] = osp.tile(
                            [128, 1024], bf, tag="osb", name=f"osb_{b}_{qp}_{h}"
                        )
                # query-block 0, head 0 — for the last batch, zip the deferred
                # tcx1 V chains in (producer safely ahead of its pv consumer:
                # mid(9+j) needs v[8+j])
                u00 = attn_chunk_units(b, 0, 0)
                if b == B - 1 and deferred_v:
                    zipped = u00[:9]  # prologue, mid(1..8)
                    for j in range(4):
                        zipped += [deferred_v[2 * j], deferred_v[2 * j + 1]]
                        zipped.append(u00[9 + j])  # mid(9+j)
                    zipped += u00[13:]  # mid(13..15), epilogue
                    u00 = zipped
                for cost, fn in u00:
                    fn()
                    yield cost
                for cost, fn in attn_chunk_units(b, 0, 1):
                    fn()
                    yield cost
                # query-block 1 attention interleaved with block-0 out-proj
                a_units = attn_chunk_units(b, 1, 0) + attn_chunk_units(b, 1, 1)
                o_units = outproj_units(b, 0)
                ca = sum(c for c, _ in a_units)
                co = sum(c for c, _ in o_units)
                sa = so = 0.0
                ia = io = 0
                while ia < len(a_units) or io < len(o_units):
                    if io >= len(o_units) or (
                        ia < len(a_units) and sa / ca <= so / co
                    ):
                        cost, fn = a_units[ia]
                        ia += 1
                        sa += cost
                    else:
                        cost, fn = o_units[io]
                        io += 1
                        so += cost
                    fn()
                    yield cost

            def unit_stream(units):
                for cost, fn in units:
                    fn()
                    yield cost

            # ---- main schedule ----
            deferred_v = []
            _interleave([qkv_stream(0)])
            prev_op = None
            for b in range(B):
                streams = [attn_stream(b)]
                if prev_op is not None:
                    streams.append(unit_stream(prev_op))
                if b < B - 1:
                    streams.append(qkv_stream(b + 1))
                _interleave(streams)
                prev_op = outproj_units(b, 1)
            for cost, fn in prev_op:
                fn()
    nc.compile()
    return nc


_NC_CACHE = None
LAST_EXEC_NS = None


def _ensure_trace_hook_stub():
    """If the image's antenv lacks axon_hooks, a stray BASS_TRACE env var
    would crash run_bass_kernel_spmd on import. Register a None-hook stub
    (concourse then logs a warning and runs without tracing)."""
    try:
        import antenv.axon_hooks  # noqa: F401
    except ImportError:
        import types

        mod = types.ModuleType("antenv.axon_hooks")
        mod.get_axon_ntff_profile_hook = lambda: None
        mod.set_axon_ntff_profile_hook = lambda h: None
        sys.modules["antenv.axon_hooks"] = mod


def kernel(**inputs):
    global _NC_CACHE, LAST_EXEC_NS
    _ensure_trace_hook_stub()
    from concourse import bass_utils

    x = np.asarray(inputs["x"], dtype=np.float32)
    w_qkv = np.asarray(inputs["w_qkv"], dtype=np.float32)
    w_out = np.asarray(inputs["w_out"], dtype=np.float32)

    bf = ml_dtypes.bfloat16
    xt_np = np.ascontiguousarray(x.transpose(0, 2, 1)).astype(bf)  # [B, E, S]

    in_maps = []
    for c in range(NCORES):
        rows_q, rows_k, rows_v = [], [], []
        for h in (HPC * c, HPC * c + 1):
            base = h * 3 * D
            rows_q.append(w_qkv[base : base + D])
            rows_k.append(w_qkv[base + D : base + 2 * D])
            rows_v.append(w_qkv[base + 2 * D : base + 3 * D])
        wqk_c = np.concatenate(rows_q + rows_k, axis=0).T  # [E, 4D]
        wv_c = np.concatenate(rows_v, axis=0).T            # [E, 2D]
        wo_c = w_out[:, HPC * c * D : (HPC * c + HPC) * D].T  # [2D, E]
        in_maps.append(
            {
                "xt": xt_np,
                "wqk": np.ascontiguousarray(wqk_c).astype(bf),
                "wv": np.ascontiguousarray(wv_c).astype(bf),
                "wo": np.ascontiguousarray(wo_c).astype(bf),
            }
        )

    if _NC_CACHE is None:
        _NC_CACHE = _build()
    nc = _NC_CACHE

    res = bass_utils.run_bass_kernel_spmd(
        nc, in_maps, core_ids=list(range(NCORES))
    )
    LAST_EXEC_NS = res.exec_time_ns

    y_t = res.results[0]["yt"].astype(np.float32)
    for c in range(1, NCORES):
        y_t += res.results[c]["yt"].astype(np.float32)
    return np.ascontiguousarray(y_t.transpose(0, 2, 1)).astype(np.float32)


# revision 23
# speedup vs baseline: 1.0083x; 1.0083x over previous
"""Multi-head attention TRN2 Bass kernel (B=4, S=2048, E=2048, H=16, D=128).

Sharding: 2 heads per core (tensor parallel over H=16 across 8 cores).
Each core computes q/k/v projections for its 2 heads over all batches,
attention, and a partial out-projection (its heads' columns of W_out).
Host sums the 8 fp32 partial outputs (the "all-reduce") and transposes.

v2 schedule: software-pipelined across batches. The QKV projection of
batch b+1 is emitted interleaved with the attention of batch b so the
in-order PE queue always has dense matmul work while the scalar engine
runs exp (attention alone is exp-bound: ~1.2us scalar vs 0.85us PE per
key-chunk). Scores run one key-chunk ahead of the attn*V matmuls, the
out-projection of query-block 0 interleaves with attention of block 1,
and the softmax denominator is a gpsimd partition-reduce instead of a
PE ones-matmul. PSUM: pq(2) + sc(4) + oc(2) = 8 banks.

Device layouts (per core):
  xt   [B, E, S]  bf16   x transposed per batch (feature-major)
  wqk  [E, 4D]    bf16   W_q/W_k columns for heads (q0|q1|k0|k1)
  wv   [E, 2D]    bf16   W_v columns (v0|v1)
  wo   [2D, E]    bf16   W_out^T rows for this core's head channels
  yt   [B, E, S]  f16    partial output, feature-major (ExternalOutput)
"""

import os
import sys

sys.path.insert(0, "/opt/trn_rl_repo")

import numpy as np
import ml_dtypes

B, S, E = 4, 2048, 2048
H, D = 16, 128
NCORES = 8
HPC = H // NCORES  # 2 heads per core
SCALE = 1.0 / float(np.sqrt(D))

EC = E // 128     # 16 contraction chunks
TCX = 1024        # x sbuf tile width (tokens)
KC = S // 128     # 16 key chunks
NQP = S // 1024   # 2 query chunk-pairs


def _interleave(streams):
    """streams: generators that emit instructions inside next() and yield
    the unit's PE cost. Alternate emission so cumulative costs stay even."""
    spent = [0.0] * len(streams)
    alive = [True] * len(streams)
    while any(alive):
        j = min((i for i in range(len(streams)) if alive[i]), key=lambda i: spent[i])
        try:
            spent[j] += next(streams[j])
        except StopIteration:
            alive[j] = False


def _build():
    import concourse.bass as bass
    import concourse.tile as tile
    from concourse import bacc, mybir

    bf = mybir.dt.bfloat16
    f16 = mybir.dt.float16
    f32 = mybir.dt.float32
    ADD = mybir.AluOpType.add
    MULT = mybir.AluOpType.mult
    EXP = mybir.ActivationFunctionType.Exp

    nc = bacc.Bacc(
        "TRN2", target_bir_lowering=False, debug=False, num_devices=NCORES
    )
    xt = nc.dram_tensor("xt", [B, E, S], bf, kind="ExternalInput").ap()
    wqk = nc.dram_tensor("wqk", [E, 4 * D], bf, kind="ExternalInput").ap()
    wv = nc.dram_tensor("wv", [E, 2 * D], bf, kind="ExternalInput").ap()
    wo = nc.dram_tensor("wo", [2 * D, E], bf, kind="ExternalInput").ap()
    yt = nc.dram_tensor("yt", [B, E, S], f16, kind="ExternalOutput").ap()

    with tile.TileContext(nc) as tc:
        with (
            tc.tile_pool(name="wp", bufs=1) as wp,
            tc.tile_pool(name="xp", bufs=20) as xp,
            tc.tile_pool(name="qkp", bufs=8) as qkp,
            tc.tile_pool(name="vp", bufs=32) as vp,
            tc.tile_pool(name="op", bufs=8) as osp,
            tc.tile_pool(name="ptp", bufs=6) as ptp,
            tc.tile_pool(name="accp", bufs=3) as accp,
            tc.tile_pool(name="dbp", bufs=2) as dbp,
            tc.tile_pool(name="yp", bufs=4) as yp,
            tc.tile_pool(name="oup", bufs=6) as oup,
            tc.tile_pool(name="psp", bufs=2, space="PSUM") as psp,
        ):
            wqk_t = [
                wp.tile([128, 4 * D], bf, tag=f"wqk{ec}", name=f"wqk_{ec}")
                for ec in range(EC)
            ]
            wv_t = [
                wp.tile([128, 2 * D], bf, tag=f"wv{ec}", name=f"wv_{ec}")
                for ec in range(EC)
            ]
            wo_t = [
                wp.tile([128, E], bf, tag=f"wo{cc}", name=f"wo_{cc}")
                for cc in range(HPC)
            ]
            ones_sb = wp.tile([128, 1], f16, tag="ones", name="ones_sb")
            nc.vector.memset(ones_sb, 1.0)

            # per-batch state, keyed b%2 for double buffering
            qk_tiles = {}
            v_tiles = {}
            out_sb = {}

            def load_wv():
                for ec in range(EC):
                    nc.scalar.dma_start(wv_t[ec], wv[ec * 128 : (ec + 1) * 128, :])

            def load_wo():
                for cc in range(HPC):
                    nc.scalar.dma_start(wo_t[cc], wo[cc * 128 : (cc + 1) * 128, :])

            def qkv_stream(b):
                """QKV projection for batch b: x DMAs + 16 QK chains + 16 V
                chains. Emits units, yields their PE cost (us)."""
                x_t = {}

                def dma_x(tcx):
                    tiles = []
                    for ec in range(EC):
                        xtile = xp.tile(
                            [128, TCX], bf, tag="x", name=f"x_{b}_{tcx}_{ec}"
                        )
                        nc.sync.dma_start(
                            xtile,
                            xt[
                                b,
                                ec * 128 : (ec + 1) * 128,
                                tcx * TCX : (tcx + 1) * TCX,
                            ],
                        )
                        tiles.append(xtile)
                    x_t[tcx] = tiles

                if b == 0:
                    # weights on the scalar DGE ring, x on the sync ring, so
                    # the first chain's operands land in parallel
                    x_t[0] = []
                    for ec in range(EC):
                        nc.scalar.dma_start(
                            wqk_t[ec], wqk[ec * 128 : (ec + 1) * 128, :]
                        )
                        xtile = xp.tile([128, TCX], bf, tag="x", name=f"x_0_0_{ec}")
                        nc.sync.dma_start(
                            xtile, xt[0, ec * 128 : (ec + 1) * 128, 0:TCX]
                        )
                        x_t[0].append(xtile)
                else:
                    dma_x(0)
                yield 0.1

                qk_tiles[b % 2] = [
                    qkp.tile([128, S], bf, tag="qk", name=f"qk_{b}_{mc}")
                    for mc in range(4)
                ]
                v_tiles[b % 2] = [
                    vp.tile([128, 2 * D], f16, tag="v", name=f"v_{b}_{kc}")
                    for kc in range(KC)
                ]

                def v_chain(tcx, tsub):
                    pv = psp.tile(
                        [128, 2 * D], f32, tag="pq", name=f"pv_{b}_{tcx}_{tsub}"
                    )
                    for ec in range(EC):
                        nc.tensor.matmul(
                            pv,
                            lhsT=x_t[tcx][ec][:, tsub * 128 : (tsub + 1) * 128],
                            rhs=wv_t[ec],
                            start=(ec == 0),
                            stop=(ec == EC - 1),
                        )
                    kc = tcx * (TCX // 128) + tsub
                    if b == 0:
                        nc.vector.tensor_copy(v_tiles[b % 2][kc], pv)
                    else:
                        nc.scalar.copy(v_tiles[b % 2][kc], pv)

                nchain = 0
                for tcx in range(S // TCX):
                    # QK chains: out [D, 512] per (mc, tcs)
                    for mc in range(4):
                        for tcs in range(2):
                            pq = psp.tile(
                                [128, 512], f32, tag="pq",
                                name=f"pq_{b}_{tcx}_{mc}_{tcs}",
                            )
                            for ec in range(EC):
                                nc.tensor.matmul(
                                    pq,
                                    lhsT=wqk_t[ec][:, mc * 128 : (mc + 1) * 128],
                                    rhs=x_t[tcx][ec][:, tcs * 512 : (tcs + 1) * 512],
                                    start=(ec == 0),
                                    stop=(ec == EC - 1),
                                )
                            tchunk = tcx * TCX + tcs * 512
                            drain = nc.vector.tensor_copy if b == 0 else nc.scalar.copy
                            drain(
                                qk_tiles[b % 2][mc][:, tchunk : tchunk + 512], pq
                            )
                            nchain += 1
                            if b == 0 and tcx == 0:
                                if nchain == 1:
                                    load_wv()
                                elif nchain == 3:
                                    load_wo()
                            if tcx == 0 and nchain == 2:
                                dma_x(1)
                            yield 3.41
                    # V chains: out [128 tokens, 2D]
                    for tsub in range(TCX // 128):
                        if b == B - 1 and tcx == 1:
                            # defer: emitted inside attn_stream(B-1) as PE
                            # filler (attention alone is exp-bound)
                            deferred_v.append(
                                (1.71, (lambda t, s: lambda: v_chain(t, s))(tcx, tsub))
                            )
                        else:
                            v_chain(tcx, tsub)
                            yield 1.71

            def attn_chunk_units(b, qp, h):
                """Attention for one (query-block, head): scores run one key
                chunk ahead of attn*V. Returns a list of (cost, fn) units."""
                q_t = qk_tiles[b % 2][h]
                k_t = qk_tiles[b % 2][2 + h]
                vt = v_tiles[b % 2]
                q0 = qp * 1024
                state = {}

                def scores(kc):
                    sps = psp.tile(
                        [128, 1024], f32, tag="sc", name=f"s_{b}_{h}_{qp}_{kc}"
                    )
                    for half in range(2):
                        nc.tensor.matmul(
                            sps[:, half * 512 : (half + 1) * 512],
                            lhsT=k_t[:, kc * 128 : (kc + 1) * 128],
                            rhs=q_t[:, q0 + half * 512 : q0 + (half + 1) * 512],
                            start=True,
                            stop=True,
                        )
                    pt = ptp.tile(
                        [128, 1024], f16, tag="pt", name=f"pt_{b}_{h}_{qp}_{kc}"
                    )
                    nc.scalar.activation(pt, sps, EXP, scale=SCALE)
                    state[kc] = pt

                def attnv(kc):
                    pt = state[kc]
                    for half, ps in ((0, state["oa"]), (1, state["ob"])):
                        nc.tensor.matmul(
                            ps,
                            lhsT=vt[kc][:, h * 128 : (h + 1) * 128],
                            rhs=pt[:, half * 512 : (half + 1) * 512],
                            start=(kc == 0),
                            stop=(kc == KC - 1),
                        )
                    acc = state["acc"]
                    if kc == 1:
                        nc.vector.tensor_tensor(acc, state[0], pt, ADD)
                    elif kc > 1:
                        nc.vector.tensor_tensor(acc, acc, pt, ADD)
                    if kc >= 1:
                        del state[kc - 1]

                def prologue():
                    state["oa"] = psp.tile(
                        [128, 512], f32, tag="oc", name=f"oa_{b}_{h}_{qp}"
                    )
                    state["ob"] = psp.tile(
                        [128, 512], f32, tag="oc", name=f"ob_{b}_{h}_{qp}"
                    )
                    state["acc"] = accp.tile(
                        [128, 1024], f16, tag="acc", name=f"acc_{b}_{h}_{qp}"
                    )
                    scores(0)

                def mid(kc):
                    def fn():
                        scores(kc)
                        attnv(kc - 1)

                    return fn

                def epilogue():
                    attnv(KC - 1)
                    ou_a = oup.tile(
                        [128, 512], f16, tag="ou", name=f"oua_{b}_{h}_{qp}"
                    )
                    ou_b = oup.tile(
                        [128, 512], f16, tag="ou", name=f"oub_{b}_{h}_{qp}"
                    )
                    nc.vector.tensor_copy(ou_a, state["oa"])
                    nc.vector.tensor_copy(ou_b, state["ob"])
                    rec = dbp.tile([1, 1024], f32, tag="rec", name=f"rec_{b}_{h}_{qp}")
                    for dh in range(2):
                        den_ps = psp.tile(
                            [1, 512], f32, tag="pq", name=f"den_{b}_{h}_{qp}_{dh}"
                        )
                        nc.tensor.matmul(
                            den_ps,
                            lhsT=ones_sb,
                            rhs=state["acc"][:, dh * 512 : (dh + 1) * 512],
                            start=True,
                            stop=True,
                        )
                        nc.vector.reciprocal_approx_fast(
                            out=rec[:, dh * 512 : (dh + 1) * 512], in_=den_ps
                        )
                    dbc = dbp.tile(
                        [128, 1024], f32, tag="dbc", name=f"dbc_{b}_{h}_{qp}"
                    )
                    nc.gpsimd.partition_broadcast(dbc, rec)
                    ot = out_sb[(b % 2, qp, h)]
                    nc.vector.tensor_tensor(ot[:, :512], ou_a, dbc[:, :512], MULT)
                    nc.vector.tensor_tensor(ot[:, 512:], ou_b, dbc[:, 512:], MULT)

                units = [(0.85, prologue)]
                units += [(0.85, mid(kc)) for kc in range(1, KC)]
                units.append((1.0, epilogue))
                return units

            def outproj_units(b, qp):
                units = []

                def fc_unit(fc):
                    def fn():
                        yps = psp.tile(
                            [128, 1024], f32, tag="sc", name=f"y_{b}_{qp}_{fc}"
                        )
                        for half in range(2):
                            for cc in range(HPC):
                                nc.tensor.matmul(
                                    yps[:, half * 512 : (half + 1) * 512],
                                    lhsT=wo_t[cc][:, fc * 128 : (fc + 1) * 128],
                                    rhs=out_sb[(b % 2, qp, cc)][
                                        :, half * 512 : (half + 1) * 512
                                    ],
                                    start=(cc == 0),
                                    stop=(cc == HPC - 1),
                                )
                        y_sb = yp.tile(
                            [128, 1024], f16, tag="y", name=f"ysb_{b}_{qp}_{fc}"
                        )
                        nc.vector.tensor_copy(y_sb, yps)
                        nc.sync.dma_start(
                            yt[
                                b,
                                fc * 128 : (fc + 1) * 128,
                                qp * 1024 : (qp + 1) * 1024,
                            ],
                            y_sb,
                        )

                    return fn

                for fc in range(E // 128):
                    units.append((0.85, fc_unit(fc)))
                return units

            def attn_stream(b):
                for qp in range(NQP):
                    for h in range(HPC):
                        out_sb[(b % 2, qp, h)] = osp.tile(
                            [128, 1024], bf, tag="osb", name=f"osb_{b}_{qp}_{h}"
                        )
                # query-block 0, head 0 — for the last batch, zip the deferred
                # tcx1 V chains in (producer safely ahead of its pv consumer:
                # mid(9+j) needs v[8+j])
                u00 = attn_chunk_units(b, 0, 0)
                if b == B - 1 and deferred_v:
                    zipped = u00[:9]  # prologue, mid(1..8)
                    for j in range(4):
                        zipped += [deferred_v[2 * j], deferred_v[2 * j + 1]]
                        zipped.append(u00[9 + j])  # mid(9+j)
                    zipped += u00[13:]  # mid(13..15), epilogue
                    u00 = zipped
                for cost, fn in u00:
                    fn()
                    yield cost
                for cost, fn in attn_chunk_units(b, 0, 1):
                    fn()
                    yield cost
                # query-block 1 attention interleaved with block-0 out-proj
                a_units = attn_chunk_units(b, 1, 0) + attn_chunk_units(b, 1, 1)
                o_units = outproj_units(b, 0)
                ca = sum(c for c, _ in a_units)
                co = sum(c for c, _ in o_units)
                sa = so = 0.0
                ia = io = 0
                while ia < len(a_units) or io < len(o_units):
                    if io >= len(o_units) or (
                        ia < len(a_units) and sa / ca <= so / co
                    ):
                        cost, fn = a_units[ia]
                        ia += 1
                        sa += cost
                    else:
                        cost, fn = o_units[io]
                        io += 1
                        so += cost
                    fn()
                    yield cost

            def unit_stream(units):
                for cost, fn in units:
                    fn()
                    yield cost

            # ---- main schedule ----
            deferred_v = []
            _interleave([qkv_stream(0)])
            prev_op = None
            for b in range(B):
                streams = [attn_stream(b)]
                if prev_op is not None:
                    streams.append(unit_stream(prev_op))
                if b < B - 1:
                    streams.append(qkv_stream(b + 1))
                _interleave(streams)
                prev_op = outproj_units(b, 1)
            for cost, fn in prev_op:
                fn()
    nc.compile()
    return nc


_NC_CACHE = None
LAST_EXEC_NS = None


def _ensure_trace_hook_stub():
    """If the image's antenv lacks axon_hooks, a stray BASS_TRACE env var
    would crash run_bass_kernel_spmd on import. Register a None-hook stub
    (concourse then logs a warning and runs without tracing)."""
    try:
        import antenv.axon_hooks  # noqa: F401
    except ImportError:
        import types

        mod = types.ModuleType("antenv.axon_hooks")
        mod.get_axon_ntff_profile_hook = lambda: None
        mod.set_axon_ntff_profile_hook = lambda h: None
        sys.modules["antenv.axon_hooks"] = mod


def kernel(**inputs):
    global _NC_CACHE, LAST_EXEC_NS
    _ensure_trace_hook_stub()
    from concourse import bass_utils

    x = np.asarray(inputs["x"], dtype=np.float32)
    w_qkv = np.asarray(inputs["w_qkv"], dtype=np.float32)
    w_out = np.asarray(inputs["w_out"], dtype=np.float32)

    bf = ml_dtypes.bfloat16
    xt_np = np.ascontiguousarray(x.transpose(0, 2, 1)).astype(bf)  # [B, E, S]

    in_maps = []
    for c in range(NCORES):
        rows_q, rows_k, rows_v = [], [], []
        for h in (HPC * c, HPC * c + 1):
            base = h * 3 * D
            rows_q.append(w_qkv[base : base + D])
            rows_k.append(w_qkv[base + D : base + 2 * D])
            rows_v.append(w_qkv[base + 2 * D : base + 3 * D])
        wqk_c = np.concatenate(rows_q + rows_k, axis=0).T  # [E, 4D]
        wv_c = np.concatenate(rows_v, axis=0).T            # [E, 2D]
        wo_c = w_out[:, HPC * c * D : (HPC * c + HPC) * D].T  # [2D, E]
        in_maps.append(
            {
                "xt": xt_np,
                "wqk": np.ascontiguousarray(wqk_c).astype(bf),
                "wv": np.ascontiguousarray(wv_c).astype(bf),
                "wo": np.ascontiguousarray(wo_c).astype(bf),
            }
        )

    if _NC_CACHE is None:
        _NC_CACHE = _build()
    nc = _NC_CACHE

    res = bass_utils.run_bass_kernel_spmd(
        nc, in_maps, core_ids=list(range(NCORES))
    )
    LAST_EXEC_NS = res.exec_time_ns

    y_t = res.results[0]["yt"].astype(np.float32)
    for c in range(1, NCORES):
        y_t += res.results[c]["yt"].astype(np.float32)
    return np.ascontiguousarray(y_t.transpose(0, 2, 1)).astype(np.float32)


# revision 28
# speedup vs baseline: 1.0289x; 1.0204x over previous
"""Multi-head attention TRN2 Bass kernel (B=4, S=2048, E=2048, H=16, D=128).

Sharding: 2 heads per core (tensor parallel over H=16 across 8 cores).
Each core computes q/k/v projections for its 2 heads over all batches,
attention, and a partial out-projection (its heads' columns of W_out).
Host sums the 8 fp32 partial outputs (the "all-reduce") and transposes.

v2 schedule: software-pipelined across batches. The QKV projection of
batch b+1 is emitted interleaved with the attention of batch b so the
in-order PE queue always has dense matmul work while the scalar engine
runs exp (attention alone is exp-bound: ~1.2us scalar vs 0.85us PE per
key-chunk). Scores run one key-chunk ahead of the attn*V matmuls, the
out-projection of query-block 0 interleaves with attention of block 1,
and the softmax denominator is a gpsimd partition-reduce instead of a
PE ones-matmul. PSUM: pq(2) + sc(4) + oc(2) = 8 banks.

Device layouts (per core):
  xt   [B, E, S]  bf16   x transposed per batch (feature-major)
  wqk  [E, 4D]    bf16   W_q/W_k columns for heads (q0|q1|k0|k1)
  wv   [E, 2D]    bf16   W_v columns (v0|v1)
  wo   [2D, E]    bf16   W_out^T rows for this core's head channels
  yt   [B, E, S]  f16    partial output, feature-major (ExternalOutput)
"""

import os
import sys

sys.path.insert(0, "/opt/trn_rl_repo")

import numpy as np
import ml_dtypes

B, S, E = 4, 2048, 2048
H, D = 16, 128
NCORES = 8
HPC = H // NCORES  # 2 heads per core
SCALE = 1.0 / float(np.sqrt(D))

EC = E // 128     # 16 contraction chunks
TCX = 1024        # x sbuf tile width (tokens)
KC = S // 128     # 16 key chunks
NQP = S // 1024   # 2 query chunk-pairs


def _interleave(streams):
    """streams: generators that emit instructions inside next() and yield
    the unit's PE cost. Alternate emission so cumulative costs stay even."""
    spent = [0.0] * len(streams)
    alive = [True] * len(streams)
    while any(alive):
        j = min((i for i in range(len(streams)) if alive[i]), key=lambda i: spent[i])
        try:
            spent[j] += next(streams[j])
        except StopIteration:
            alive[j] = False


def _build():
    import concourse.bass as bass
    import concourse.tile as tile
    from concourse import bacc, mybir

    bf = mybir.dt.bfloat16
    f16 = mybir.dt.float16
    f32 = mybir.dt.float32
    ADD = mybir.AluOpType.add
    MULT = mybir.AluOpType.mult
    EXP = mybir.ActivationFunctionType.Exp

    nc = bacc.Bacc(
        "TRN2", target_bir_lowering=False, debug=False, num_devices=NCORES
    )
    xt = nc.dram_tensor("xt", [B, E, S], bf, kind="ExternalInput").ap()
    wqk = nc.dram_tensor("wqk", [E, 4 * D], bf, kind="ExternalInput").ap()
    wv = nc.dram_tensor("wv", [E, 2 * D], bf, kind="ExternalInput").ap()
    wo = nc.dram_tensor("wo", [2 * D, E], bf, kind="ExternalInput").ap()
    yt = nc.dram_tensor("yt", [B, E, S], f16, kind="ExternalOutput").ap()

    with tile.TileContext(nc) as tc:
        with (
            tc.tile_pool(name="wp", bufs=1) as wp,
            tc.tile_pool(name="xp", bufs=20) as xp,
            tc.tile_pool(name="qkp", bufs=8) as qkp,
            tc.tile_pool(name="vp", bufs=32) as vp,
            tc.tile_pool(name="op", bufs=8) as osp,
            tc.tile_pool(name="ptp", bufs=6) as ptp,
            tc.tile_pool(name="accp", bufs=3) as accp,
            tc.tile_pool(name="dbp", bufs=2) as dbp,
            tc.tile_pool(name="yp", bufs=4) as yp,
            tc.tile_pool(name="oup", bufs=6) as oup,
            tc.tile_pool(name="psp", bufs=2, space="PSUM") as psp,
        ):
            wqk_t = [
                wp.tile([128, 4 * D], bf, tag=f"wqk{ec}", name=f"wqk_{ec}")
                for ec in range(EC)
            ]
            wv_t = [
                wp.tile([128, 2 * D], bf, tag=f"wv{ec}", name=f"wv_{ec}")
                for ec in range(EC)
            ]
            wo_t = [
                wp.tile([128, E], bf, tag=f"wo{cc}", name=f"wo_{cc}")
                for cc in range(HPC)
            ]
            ones_sb = wp.tile([128, 1], f16, tag="ones", name="ones_sb")
            nc.vector.memset(ones_sb, 1.0)

            # per-batch state, keyed b%2 for double buffering
            qk_tiles = {}
            v_tiles = {}
            out_sb = {}

            def load_wv():
                for ec in range(EC):
                    nc.scalar.dma_start(wv_t[ec], wv[ec * 128 : (ec + 1) * 128, :])

            def load_wo():
                for cc in range(HPC):
                    nc.scalar.dma_start(wo_t[cc], wo[cc * 128 : (cc + 1) * 128, :])

            def qkv_stream(b):
                """QKV projection for batch b: x DMAs + 16 QK chains + 16 V
                chains. Emits units, yields their PE cost (us)."""
                x_t = {}

                def dma_x(tcx):
                    tiles = []
                    for ec in range(EC):
                        xtile = xp.tile(
                            [128, TCX], bf, tag="x", name=f"x_{b}_{tcx}_{ec}"
                        )
                        nc.sync.dma_start(
                            xtile,
                            xt[
                                b,
                                ec * 128 : (ec + 1) * 128,
                                tcx * TCX : (tcx + 1) * TCX,
                            ],
                        )
                        tiles.append(xtile)
                    x_t[tcx] = tiles

                if b == 0:
                    # weights on the scalar DGE ring, x on the sync ring, so
                    # the first chain's operands land in parallel
                    x_t[0] = []
                    for ec in range(EC):
                        nc.scalar.dma_start(
                            wqk_t[ec], wqk[ec * 128 : (ec + 1) * 128, :]
                        )
                        xtile = xp.tile([128, TCX], bf, tag="x", name=f"x_0_0_{ec}")
                        nc.sync.dma_start(
                            xtile, xt[0, ec * 128 : (ec + 1) * 128, 0:TCX]
                        )
                        x_t[0].append(xtile)
                else:
                    dma_x(0)
                yield 0.1

                qk_tiles[b % 2] = [
                    qkp.tile([128, S], bf, tag="qk", name=f"qk_{b}_{mc}")
                    for mc in range(4)
                ]
                v_tiles[b % 2] = [
                    vp.tile([128, 2 * D], f16, tag="v", name=f"v_{b}_{kc}")
                    for kc in range(KC)
                ]

                def v_chain(tcx, tsub):
                    pv = psp.tile(
                        [128, 2 * D], f32, tag="pq", name=f"pv_{b}_{tcx}_{tsub}"
                    )
                    for ec in range(EC):
                        nc.tensor.matmul(
                            pv,
                            lhsT=x_t[tcx][ec][:, tsub * 128 : (tsub + 1) * 128],
                            rhs=wv_t[ec],
                            start=(ec == 0),
                            stop=(ec == EC - 1),
                        )
                    kc = tcx * (TCX // 128) + tsub
                    nc.scalar.copy(v_tiles[b % 2][kc], pv)

                def v_chain_half(tcx, tsub, hf):
                    if hf == 0:
                        state_v["pv"] = psp.tile(
                            [128, 2 * D], f32, tag="pq", name=f"pv_{b}_{tcx}_{tsub}"
                        )
                    pv = state_v["pv"]
                    for ec in range(hf * 8, hf * 8 + 8):
                        nc.tensor.matmul(
                            pv,
                            lhsT=x_t[tcx][ec][:, tsub * 128 : (tsub + 1) * 128],
                            rhs=wv_t[ec],
                            start=(ec == 0),
                            stop=(ec == EC - 1),
                        )
                    if hf == 1:
                        kc = tcx * (TCX // 128) + tsub
                        nc.scalar.copy(v_tiles[b % 2][kc], state_v["pv"])

                state_v = {}

                nchain = 0
                for tcx in range(S // TCX):
                    # QK chains: out [D, 512] per (mc, tcs), emitted as two
                    # 8-matmul half-units for finer interleave granularity
                    for mc in range(4):
                        for tcs in range(2):
                            pq = psp.tile(
                                [128, 512], f32, tag="pq",
                                name=f"pq_{b}_{tcx}_{mc}_{tcs}",
                            )
                            for ec in range(8):
                                nc.tensor.matmul(
                                    pq,
                                    lhsT=wqk_t[ec][:, mc * 128 : (mc + 1) * 128],
                                    rhs=x_t[tcx][ec][:, tcs * 512 : (tcs + 1) * 512],
                                    start=(ec == 0),
                                    stop=False,
                                )
                            yield 1.71
                            for ec in range(8, EC):
                                nc.tensor.matmul(
                                    pq,
                                    lhsT=wqk_t[ec][:, mc * 128 : (mc + 1) * 128],
                                    rhs=x_t[tcx][ec][:, tcs * 512 : (tcs + 1) * 512],
                                    start=False,
                                    stop=(ec == EC - 1),
                                )
                            tchunk = tcx * TCX + tcs * 512
                            nc.scalar.copy(
                                qk_tiles[b % 2][mc][:, tchunk : tchunk + 512], pq
                            )
                            nchain += 1
                            if b == 0 and tcx == 0:
                                if nchain == 1:
                                    load_wv()
                                elif nchain == 3:
                                    load_wo()
                            if tcx == 0 and nchain == 2:
                                dma_x(1)
                            yield 1.71
                    # V chains: out [128 tokens, 2D]
                    for tsub in range(TCX // 128):
                        if b == B - 1 and tcx == 1:
                            # defer: emitted inside attn_stream(B-1) as PE
                            # filler (attention alone is exp-bound)
                            deferred_v.append(
                                (1.71, (lambda t, s: lambda: v_chain(t, s))(tcx, tsub))
                            )
                        else:
                            for hf in range(2):
                                v_chain_half(tcx, tsub, hf)
                                yield 0.85

            def attn_chunk_units(b, qp, h):
                """Attention for one (query-block, head): scores run one key
                chunk ahead of attn*V. Returns a list of (cost, fn) units."""
                q_t = qk_tiles[b % 2][h]
                k_t = qk_tiles[b % 2][2 + h]
                vt = v_tiles[b % 2]
                q0 = qp * 1024
                state = {}

                def scores(kc):
                    sps = psp.tile(
                        [128, 1024], f32, tag="sc", name=f"s_{b}_{h}_{qp}_{kc}"
                    )
                    for half in range(2):
                        nc.tensor.matmul(
                            sps[:, half * 512 : (half + 1) * 512],
                            lhsT=k_t[:, kc * 128 : (kc + 1) * 128],
                            rhs=q_t[:, q0 + half * 512 : q0 + (half + 1) * 512],
                            start=True,
                            stop=True,
                        )
                    pt = ptp.tile(
                        [128, 1024], f16, tag="pt", name=f"pt_{b}_{h}_{qp}_{kc}"
                    )
                    nc.scalar.activation(pt, sps, EXP, scale=SCALE)
                    state[kc] = pt

                def attnv(kc):
                    pt = state[kc]
                    for half, ps in ((0, state["oa"]), (1, state["ob"])):
                        nc.tensor.matmul(
                            ps,
                            lhsT=vt[kc][:, h * 128 : (h + 1) * 128],
                            rhs=pt[:, half * 512 : (half + 1) * 512],
                            start=(kc == 0),
                            stop=(kc == KC - 1),
                        )
                    acc = state["acc"]
                    if kc == 1:
                        nc.vector.tensor_tensor(acc, state[0], pt, ADD)
                    elif kc > 1:
                        nc.vector.tensor_tensor(acc, acc, pt, ADD)
                    if kc >= 1:
                        del state[kc - 1]

                def prologue():
                    state["oa"] = psp.tile(
                        [128, 512], f32, tag="oc", name=f"oa_{b}_{h}_{qp}"
                    )
                    state["ob"] = psp.tile(
                        [128, 512], f32, tag="oc", name=f"ob_{b}_{h}_{qp}"
                    )
                    state["acc"] = accp.tile(
                        [128, 1024], f16, tag="acc", name=f"acc_{b}_{h}_{qp}"
                    )
                    scores(0)

                def mid(kc):
                    def fn():
                        scores(kc)
                        attnv(kc - 1)

                    return fn

                def epilogue():
                    attnv(KC - 1)
                    ou_a = oup.tile(
                        [128, 512], f16, tag="ou", name=f"oua_{b}_{h}_{qp}"
                    )
                    ou_b = oup.tile(
                        [128, 512], f16, tag="ou", name=f"oub_{b}_{h}_{qp}"
                    )
                    nc.vector.tensor_copy(ou_a, state["oa"])
                    nc.vector.tensor_copy(ou_b, state["ob"])
                    rec = dbp.tile([1, 1024], f32, tag="rec", name=f"rec_{b}_{h}_{qp}")
                    for dh in range(2):
                        den_ps = psp.tile(
                            [1, 512], f32, tag="oc", name=f"den_{b}_{h}_{qp}_{dh}"
                        )
                        nc.tensor.matmul(
                            den_ps,
                            lhsT=ones_sb,
                            rhs=state["acc"][:, dh * 512 : (dh + 1) * 512],
                            start=True,
                            stop=True,
                        )
                        nc.vector.reciprocal_approx_fast(
                            out=rec[:, dh * 512 : (dh + 1) * 512], in_=den_ps
                        )
                    dbc = dbp.tile(
                        [128, 1024], f32, tag="dbc", name=f"dbc_{b}_{h}_{qp}"
                    )
                    nc.gpsimd.partition_broadcast(dbc, rec)
                    ot = out_sb[(b % 2, qp, h)]
                    nc.vector.tensor_tensor(ot[:, :512], ou_a, dbc[:, :512], MULT)
                    nc.vector.tensor_tensor(ot[:, 512:], ou_b, dbc[:, 512:], MULT)

                units = [(0.85, prologue)]
                units += [(0.85, mid(kc)) for kc in range(1, KC)]
                units.append((1.0, epilogue))
                return units

            def outproj_units(b, qp):
                units = []

                def fc_unit(fc):
                    def fn():
                        yps = psp.tile(
                            [128, 1024], f32, tag="sc", name=f"y_{b}_{qp}_{fc}"
                        )
                        for half in range(2):
                            for cc in range(HPC):
                                nc.tensor.matmul(
                                    yps[:, half * 512 : (half + 1) * 512],
                                    lhsT=wo_t[cc][:, fc * 128 : (fc + 1) * 128],
                                    rhs=out_sb[(b % 2, qp, cc)][
                                        :, half * 512 : (half + 1) * 512
                                    ],
                                    start=(cc == 0),
                                    stop=(cc == HPC - 1),
                                )
                        y_sb = yp.tile(
                            [128, 1024], f16, tag="y", name=f"ysb_{b}_{qp}_{fc}"
                        )
                        nc.vector.tensor_copy(y_sb, yps)
                        nc.sync.dma_start(
                            yt[
                                b,
                                fc * 128 : (fc + 1) * 128,
                                qp * 1024 : (qp + 1) * 1024,
                            ],
                            y_sb,
                        )

                    return fn

                for fc in range(E // 128):
                    units.append((0.85, fc_unit(fc)))
                return units

            def attn_stream(b):
                for qp in range(NQP):
                    for h in range(HPC):
                        out_sb[(b % 2, qp, h)] = osp.tile(
                            [128, 1024], bf, tag="osb", name=f"osb_{b}_{qp}_{h}"
                        )
                # query-block 0, head 0 — for the last batch, zip the deferred
                # tcx1 V chains in (producer safely ahead of its pv consumer:
                # mid(9+j) needs v[8+j])
                u00 = attn_chunk_units(b, 0, 0)
                if b == B - 1 and deferred_v:
                    zipped = u00[:9]  # prologue, mid(1..8)
                    for j in range(4):
                        zipped += [deferred_v[2 * j], deferred_v[2 * j + 1]]
                        zipped.append(u00[9 + j])  # mid(9+j)
                    zipped += u00[13:]  # mid(13..15), epilogue
                    u00 = zipped
                for cost, fn in u00:
                    fn()
                    yield cost
                for cost, fn in attn_chunk_units(b, 0, 1):
                    fn()
                    yield cost
                # query-block 1 attention interleaved with block-0 out-proj
                a_units = attn_chunk_units(b, 1, 0) + attn_chunk_units(b, 1, 1)
                o_units = outproj_units(b, 0)
                ca = sum(c for c, _ in a_units)
                co = sum(c for c, _ in o_units)
                sa = so = 0.0
                ia = io = 0
                while ia < len(a_units) or io < len(o_units):
                    if io >= len(o_units) or (
                        ia < len(a_units) and sa / ca <= so / co
                    ):
                        cost, fn = a_units[ia]
                        ia += 1
                        sa += cost
                    else:
                        cost, fn = o_units[io]
                        io += 1
                        so += cost
                    fn()
                    yield cost

            def unit_stream(units):
                for cost, fn in units:
                    fn()
                    yield cost

            # ---- main schedule ----
            deferred_v = []
            _interleave([qkv_stream(0)])
            prev_op = None
            for b in range(B):
                streams = [attn_stream(b)]
                if prev_op is not None:
                    streams.append(unit_stream(prev_op))
                if b < B - 1:
                    streams.append(qkv_stream(b + 1))
                _interleave(streams)
                prev_op = outproj_units(b, 1)
            for cost, fn in prev_op:
                fn()
    nc.compile()
    return nc


_NC_CACHE = None
LAST_EXEC_NS = None


def _ensure_trace_hook_stub():
    """If the image's antenv lacks axon_hooks, a stray BASS_TRACE env var
    would crash run_bass_kernel_spmd on import. Register a None-hook stub
    (concourse then logs a warning and runs without tracing)."""
    try:
        import antenv.axon_hooks  # noqa: F401
    except ImportError:
        import types

        mod = types.ModuleType("antenv.axon_hooks")
        mod.get_axon_ntff_profile_hook = lambda: None
        mod.set_axon_ntff_profile_hook = lambda h: None
        sys.modules["antenv.axon_hooks"] = mod


def kernel(**inputs):
    global _NC_CACHE, LAST_EXEC_NS
    _ensure_trace_hook_stub()
    from concourse import bass_utils

    x = np.asarray(inputs["x"], dtype=np.float32)
    w_qkv = np.asarray(inputs["w_qkv"], dtype=np.float32)
    w_out = np.asarray(inputs["w_out"], dtype=np.float32)

    bf = ml_dtypes.bfloat16
    xt_np = np.ascontiguousarray(x.transpose(0, 2, 1)).astype(bf)  # [B, E, S]

    in_maps = []
    for c in range(NCORES):
        rows_q, rows_k, rows_v = [], [], []
        for h in (HPC * c, HPC * c + 1):
            base = h * 3 * D
            rows_q.append(w_qkv[base : base + D])
            rows_k.append(w_qkv[base + D : base + 2 * D])
            rows_v.append(w_qkv[base + 2 * D : base + 3 * D])
        wqk_c = np.concatenate(rows_q + rows_k, axis=0).T  # [E, 4D]
        wv_c = np.concatenate(rows_v, axis=0).T            # [E, 2D]
        wo_c = w_out[:, HPC * c * D : (HPC * c + HPC) * D].T  # [2D, E]
        in_maps.append(
            {
                "xt": xt_np,
                "wqk": np.ascontiguousarray(wqk_c).astype(bf),
                "wv": np.ascontiguousarray(wv_c).astype(bf),
                "wo": np.ascontiguousarray(wo_c).astype(bf),
            }
        )

    if _NC_CACHE is None:
        _NC_CACHE = _build()
    nc = _NC_CACHE

    res = bass_utils.run_bass_kernel_spmd(
        nc, in_maps, core_ids=list(range(NCORES))
    )
    LAST_EXEC_NS = res.exec_time_ns

    y_t = res.results[0]["yt"].astype(np.float32)
    for c in range(1, NCORES):
        y_t += res.results[c]["yt"].astype(np.float32)
    return np.ascontiguousarray(y_t.transpose(0, 2, 1)).astype(np.float32)


# revision 30
# speedup vs baseline: 1.0294x; 1.0005x over previous
"""Multi-head attention TRN2 Bass kernel (B=4, S=2048, E=2048, H=16, D=128).

Sharding: 2 heads per core (tensor parallel over H=16 across 8 cores).
Each core computes q/k/v projections for its 2 heads over all batches,
attention, and a partial out-projection (its heads' columns of W_out).
Host sums the 8 fp32 partial outputs (the "all-reduce") and transposes.

v2 schedule: software-pipelined across batches. The QKV projection of
batch b+1 is emitted interleaved with the attention of batch b so the
in-order PE queue always has dense matmul work while the scalar engine
runs exp (attention alone is exp-bound: ~1.2us scalar vs 0.85us PE per
key-chunk). Scores run one key-chunk ahead of the attn*V matmuls, the
out-projection of query-block 0 interleaves with attention of block 1,
and the softmax denominator is a gpsimd partition-reduce instead of a
PE ones-matmul. PSUM: pq(2) + sc(4) + oc(2) = 8 banks.

Device layouts (per core):
  xt   [B, E, S]  bf16   x transposed per batch (feature-major)
  wqk  [E, 4D]    bf16   W_q/W_k columns for heads (q0|q1|k0|k1)
  wv   [E, 2D]    bf16   W_v columns (v0|v1)
  wo   [2D, E]    bf16   W_out^T rows for this core's head channels
  yt   [B, E, S]  f16    partial output, feature-major (ExternalOutput)
"""

import os
import sys

sys.path.insert(0, "/opt/trn_rl_repo")

import numpy as np
import ml_dtypes

B, S, E = 4, 2048, 2048
H, D = 16, 128
NCORES = 8
HPC = H // NCORES  # 2 heads per core
SCALE = 1.0 / float(np.sqrt(D))

EC = E // 128     # 16 contraction chunks
TCX = 1024        # x sbuf tile width (tokens)
KC = S // 128     # 16 key chunks
NQP = S // 1024   # 2 query chunk-pairs


def _interleave(streams):
    """streams: generators that emit instructions inside next() and yield
    the unit's PE cost. Alternate emission so cumulative costs stay even."""
    spent = [0.0] * len(streams)
    alive = [True] * len(streams)
    while any(alive):
        j = min((i for i in range(len(streams)) if alive[i]), key=lambda i: spent[i])
        try:
            spent[j] += next(streams[j])
        except StopIteration:
            alive[j] = False


def _build():
    import concourse.bass as bass
    import concourse.tile as tile
    from concourse import bacc, mybir

    bf = mybir.dt.bfloat16
    f16 = mybir.dt.float16
    f32 = mybir.dt.float32
    ADD = mybir.AluOpType.add
    MULT = mybir.AluOpType.mult
    EXP = mybir.ActivationFunctionType.Exp

    nc = bacc.Bacc(
        "TRN2", target_bir_lowering=False, debug=False, num_devices=NCORES
    )
    xt = nc.dram_tensor("xt", [B, E, S], bf, kind="ExternalInput").ap()
    wqk = nc.dram_tensor("wqk", [E, 4 * D], bf, kind="ExternalInput").ap()
    wv = nc.dram_tensor("wv", [E, 2 * D], bf, kind="ExternalInput").ap()
    wo = nc.dram_tensor("wo", [2 * D, E], bf, kind="ExternalInput").ap()
    yt = nc.dram_tensor("yt", [B, E, S], f16, kind="ExternalOutput").ap()

    with tile.TileContext(nc) as tc:
        with (
            tc.tile_pool(name="wp", bufs=1) as wp,
            tc.tile_pool(name="xp", bufs=20) as xp,
            tc.tile_pool(name="qkp", bufs=8) as qkp,
            tc.tile_pool(name="vp", bufs=32) as vp,
            tc.tile_pool(name="op", bufs=8) as osp,
            tc.tile_pool(name="ptp", bufs=6) as ptp,
            tc.tile_pool(name="accp", bufs=3) as accp,
            tc.tile_pool(name="dbp", bufs=2) as dbp,
            tc.tile_pool(name="yp", bufs=4) as yp,
            tc.tile_pool(name="oup", bufs=6) as oup,
            tc.tile_pool(name="psp", bufs=2, space="PSUM") as psp,
        ):
            wqk_t = [
                wp.tile([128, 4 * D], bf, tag=f"wqk{ec}", name=f"wqk_{ec}")
                for ec in range(EC)
            ]
            wv_t = [
                wp.tile([128, 2 * D], bf, tag=f"wv{ec}", name=f"wv_{ec}")
                for ec in range(EC)
            ]
            wo_t = [
                wp.tile([128, E], bf, tag=f"wo{cc}", name=f"wo_{cc}")
                for cc in range(HPC)
            ]
            ones_sb = wp.tile([128, 1], f16, tag="ones", name="ones_sb")
            nc.vector.memset(ones_sb, 1.0)

            # per-batch state, keyed b%2 for double buffering
            qk_tiles = {}
            v_tiles = {}
            out_sb = {}

            def load_wv():
                for ec in range(EC):
                    nc.scalar.dma_start(wv_t[ec], wv[ec * 128 : (ec + 1) * 128, :])

            def load_wo():
                for cc in range(HPC):
                    nc.scalar.dma_start(wo_t[cc], wo[cc * 128 : (cc + 1) * 128, :])

            def qkv_stream(b):
                """QKV projection for batch b: x DMAs + 16 QK chains + 16 V
                chains. Emits units, yields their PE cost (us)."""
                x_t = {}

                def dma_x(tcx):
                    tiles = []
                    for ec in range(EC):
                        xtile = xp.tile(
                            [128, TCX], bf, tag="x", name=f"x_{b}_{tcx}_{ec}"
                        )
                        nc.sync.dma_start(
                            xtile,
                            xt[
                                b,
                                ec * 128 : (ec + 1) * 128,
                                tcx * TCX : (tcx + 1) * TCX,
                            ],
                        )
                        tiles.append(xtile)
                    x_t[tcx] = tiles

                if b == 0:
                    # weights on the scalar DGE ring, x on the sync ring, so
                    # the first chain's operands land in parallel
                    x_t[0] = []
                    for ec in range(EC):
                        nc.scalar.dma_start(
                            wqk_t[ec], wqk[ec * 128 : (ec + 1) * 128, :]
                        )
                        xtile = xp.tile([128, TCX], bf, tag="x", name=f"x_0_0_{ec}")
                        nc.sync.dma_start(
                            xtile, xt[0, ec * 128 : (ec + 1) * 128, 0:TCX]
                        )
                        x_t[0].append(xtile)
                else:
                    dma_x(0)
                yield 0.1

                qk_tiles[b % 2] = [
                    qkp.tile([128, S], bf, tag="qk", name=f"qk_{b}_{mc}")
                    for mc in range(4)
                ]
                v_tiles[b % 2] = [
                    vp.tile([128, 2 * D], f16, tag="v", name=f"v_{b}_{kc}")
                    for kc in range(KC)
                ]

                def v_chain(tcx, tsub):
                    pv = psp.tile(
                        [128, 2 * D], f32, tag="pq", name=f"pv_{b}_{tcx}_{tsub}"
                    )
                    for ec in range(EC):
                        nc.tensor.matmul(
                            pv,
                            lhsT=x_t[tcx][ec][:, tsub * 128 : (tsub + 1) * 128],
                            rhs=wv_t[ec],
                            start=(ec == 0),
                            stop=(ec == EC - 1),
                        )
                    kc = tcx * (TCX // 128) + tsub
                    nc.scalar.copy(v_tiles[b % 2][kc], pv)

                def v_chain_half(tcx, tsub, hf):
                    if hf == 0:
                        state_v["pv"] = psp.tile(
                            [128, 2 * D], f32, tag="pq", name=f"pv_{b}_{tcx}_{tsub}"
                        )
                    pv = state_v["pv"]
                    for ec in range(hf * 8, hf * 8 + 8):
                        nc.tensor.matmul(
                            pv,
                            lhsT=x_t[tcx][ec][:, tsub * 128 : (tsub + 1) * 128],
                            rhs=wv_t[ec],
                            start=(ec == 0),
                            stop=(ec == EC - 1),
                        )
                    if hf == 1:
                        kc = tcx * (TCX // 128) + tsub
                        nc.scalar.copy(v_tiles[b % 2][kc], state_v["pv"])

                state_v = {}

                nchain = 0
                for tcx in range(S // TCX):
                    # QK chains: out [D, 512] per (mc, tcs), emitted as two
                    # 8-matmul half-units for finer interleave granularity
                    for mc in range(4):
                        for tcs in range(2):
                            pq = psp.tile(
                                [128, 512], f32, tag="pq",
                                name=f"pq_{b}_{tcx}_{mc}_{tcs}",
                            )
                            for q4 in range(4):
                                for ec in range(q4 * 4, q4 * 4 + 4):
                                    nc.tensor.matmul(
                                        pq,
                                        lhsT=wqk_t[ec][:, mc * 128 : (mc + 1) * 128],
                                        rhs=x_t[tcx][ec][:, tcs * 512 : (tcs + 1) * 512],
                                        start=(ec == 0),
                                        stop=(ec == EC - 1),
                                    )
                                if q4 < 3:
                                    yield 0.85
                            tchunk = tcx * TCX + tcs * 512
                            nc.scalar.copy(
                                qk_tiles[b % 2][mc][:, tchunk : tchunk + 512], pq
                            )
                            nchain += 1
                            if b == 0 and tcx == 0:
                                if nchain == 1:
                                    load_wv()
                                elif nchain == 3:
                                    load_wo()
                            if tcx == 0 and nchain == 2:
                                dma_x(1)
                            yield 0.85
                    # V chains: out [128 tokens, 2D]
                    for tsub in range(TCX // 128):
                        if b == B - 1 and tcx == 1:
                            # defer: emitted inside attn_stream(B-1) as PE
                            # filler (attention alone is exp-bound)
                            deferred_v.append(
                                (1.71, (lambda t, s: lambda: v_chain(t, s))(tcx, tsub))
                            )
                        else:
                            for hf in range(2):
                                v_chain_half(tcx, tsub, hf)
                                yield 0.85

            def attn_chunk_units(b, qp, h):
                """Attention for one (query-block, head): scores run one key
                chunk ahead of attn*V. Returns a list of (cost, fn) units."""
                q_t = qk_tiles[b % 2][h]
                k_t = qk_tiles[b % 2][2 + h]
                vt = v_tiles[b % 2]
                q0 = qp * 1024
                state = {}

                def scores(kc):
                    sps = psp.tile(
                        [128, 1024], f32, tag="sc", name=f"s_{b}_{h}_{qp}_{kc}"
                    )
                    for half in range(2):
                        nc.tensor.matmul(
                            sps[:, half * 512 : (half + 1) * 512],
                            lhsT=k_t[:, kc * 128 : (kc + 1) * 128],
                            rhs=q_t[:, q0 + half * 512 : q0 + (half + 1) * 512],
                            start=True,
                            stop=True,
                        )
                    pt = ptp.tile(
                        [128, 1024], f16, tag="pt", name=f"pt_{b}_{h}_{qp}_{kc}"
                    )
                    nc.scalar.activation(pt, sps, EXP, scale=SCALE)
                    state[kc] = pt

                def attnv(kc):
                    pt = state[kc]
                    for half, ps in ((0, state["oa"]), (1, state["ob"])):
                        nc.tensor.matmul(
                            ps,
                            lhsT=vt[kc][:, h * 128 : (h + 1) * 128],
                            rhs=pt[:, half * 512 : (half + 1) * 512],
                            start=(kc == 0),
                            stop=(kc == KC - 1),
                        )
                    acc = state["acc"]
                    if kc == 1:
                        nc.vector.tensor_tensor(acc, state[0], pt, ADD)
                    elif kc > 1:
                        nc.vector.tensor_tensor(acc, acc, pt, ADD)
                    if kc >= 1:
                        del state[kc - 1]

                def prologue():
                    state["oa"] = psp.tile(
                        [128, 512], f32, tag="oc", name=f"oa_{b}_{h}_{qp}"
                    )
                    state["ob"] = psp.tile(
                        [128, 512], f32, tag="oc", name=f"ob_{b}_{h}_{qp}"
                    )
                    state["acc"] = accp.tile(
                        [128, 1024], f16, tag="acc", name=f"acc_{b}_{h}_{qp}"
                    )
                    scores(0)

                def mid(kc):
                    def fn():
                        scores(kc)
                        attnv(kc - 1)

                    return fn

                def epilogue():
                    attnv(KC - 1)
                    ou_a = oup.tile(
                        [128, 512], f16, tag="ou", name=f"oua_{b}_{h}_{qp}"
                    )
                    ou_b = oup.tile(
                        [128, 512], f16, tag="ou", name=f"oub_{b}_{h}_{qp}"
                    )
                    nc.vector.tensor_copy(ou_a, state["oa"])
                    nc.vector.tensor_copy(ou_b, state["ob"])
                    rec = dbp.tile([1, 1024], f32, tag="rec", name=f"rec_{b}_{h}_{qp}")
                    for dh in range(2):
                        den_ps = psp.tile(
                            [1, 512], f32, tag="oc", name=f"den_{b}_{h}_{qp}_{dh}"
                        )
                        nc.tensor.matmul(
                            den_ps,
                            lhsT=ones_sb,
                            rhs=state["acc"][:, dh * 512 : (dh + 1) * 512],
                            start=True,
                            stop=True,
                        )
                        nc.vector.reciprocal_approx_fast(
                            out=rec[:, dh * 512 : (dh + 1) * 512], in_=den_ps
                        )
                    dbc = dbp.tile(
                        [128, 1024], f32, tag="dbc", name=f"dbc_{b}_{h}_{qp}"
                    )
                    nc.gpsimd.partition_broadcast(dbc, rec)
                    ot = out_sb[(b % 2, qp, h)]
                    nc.vector.tensor_tensor(ot[:, :512], ou_a, dbc[:, :512], MULT)
                    nc.vector.tensor_tensor(ot[:, 512:], ou_b, dbc[:, 512:], MULT)

                units = [(0.85, prologue)]
                units += [(0.85, mid(kc)) for kc in range(1, KC)]
                units.append((1.0, epilogue))
                return units

            def outproj_units(b, qp):
                units = []

                def fc_unit(fc):
                    def fn():
                        yps = psp.tile(
                            [128, 1024], f32, tag="sc", name=f"y_{b}_{qp}_{fc}"
                        )
                        for half in range(2):
                            for cc in range(HPC):
                                nc.tensor.matmul(
                                    yps[:, half * 512 : (half + 1) * 512],
                                    lhsT=wo_t[cc][:, fc * 128 : (fc + 1) * 128],
                                    rhs=out_sb[(b % 2, qp, cc)][
                                        :, half * 512 : (half + 1) * 512
                                    ],
                                    start=(cc == 0),
                                    stop=(cc == HPC - 1),
                                )
                        y_sb = yp.tile(
                            [128, 1024], f16, tag="y", name=f"ysb_{b}_{qp}_{fc}"
                        )
                        nc.vector.tensor_copy(y_sb, yps)
                        nc.sync.dma_start(
                            yt[
                                b,
                                fc * 128 : (fc + 1) * 128,
                                qp * 1024 : (qp + 1) * 1024,
                            ],
                            y_sb,
                        )

                    return fn

                for fc in range(E // 128):
                    units.append((0.85, fc_unit(fc)))
                return units

            def attn_stream(b):
                for qp in range(NQP):
                    for h in range(HPC):
                        out_sb[(b % 2, qp, h)] = osp.tile(
                            [128, 1024], bf, tag="osb", name=f"osb_{b}_{qp}_{h}"
                        )
                # query-block 0, head 0 — for the last batch, zip the deferred
                # tcx1 V chains in (producer safely ahead of its pv consumer:
                # mid(9+j) needs v[8+j])
                u00 = attn_chunk_units(b, 0, 0)
                if b == B - 1 and deferred_v:
                    zipped = u00[:9]  # prologue, mid(1..8)
                    for j in range(4):
                        zipped += [deferred_v[2 * j], deferred_v[2 * j + 1]]
                        zipped.append(u00[9 + j])  # mid(9+j)
                    zipped += u00[13:]  # mid(13..15), epilogue
                    u00 = zipped
                for cost, fn in u00:
                    fn()
                    yield cost
                for cost, fn in attn_chunk_units(b, 0, 1):
                    fn()
                    yield cost
                # query-block 1 attention interleaved with block-0 out-proj
                a_units = attn_chunk_units(b, 1, 0) + attn_chunk_units(b, 1, 1)
                o_units = outproj_units(b, 0)
                ca = sum(c for c, _ in a_units)
                co = sum(c for c, _ in o_units)
                sa = so = 0.0
                ia = io = 0
                while ia < len(a_units) or io < len(o_units):
                    if io >= len(o_units) or (
                        ia < len(a_units) and sa / ca <= so / co
                    ):
                        cost, fn = a_units[ia]
                        ia += 1
                        sa += cost
                    else:
                        cost, fn = o_units[io]
                        io += 1
                        so += cost
                    fn()
                    yield cost

            def unit_stream(units):
                for cost, fn in units:
                    fn()
                    yield cost

            # ---- main schedule ----
            deferred_v = []
            _interleave([qkv_stream(0)])
            prev_op = None
            for b in range(B):
                streams = [attn_stream(b)]
                if prev_op is not None:
                    streams.append(unit_stream(prev_op))
                if b < B - 1:
                    streams.append(qkv_stream(b + 1))
                _interleave(streams)
                prev_op = outproj_units(b, 1)
            for cost, fn in prev_op:
                fn()
    nc.compile()
    return nc


_NC_CACHE = None
LAST_EXEC_NS = None


def _ensure_trace_hook_stub():
    """If the image's antenv lacks axon_hooks, a stray BASS_TRACE env var
    would crash run_bass_kernel_spmd on import. Register a None-hook stub
    (concourse then logs a warning and runs without tracing)."""
    try:
        import antenv.axon_hooks  # noqa: F401
    except ImportError:
        import types

        mod = types.ModuleType("antenv.axon_hooks")
        mod.get_axon_ntff_profile_hook = lambda: None
        mod.set_axon_ntff_profile_hook = lambda h: None
        sys.modules["antenv.axon_hooks"] = mod


def kernel(**inputs):
    global _NC_CACHE, LAST_EXEC_NS
    _ensure_trace_hook_stub()
    from concourse import bass_utils

    x = np.asarray(inputs["x"], dtype=np.float32)
    w_qkv = np.asarray(inputs["w_qkv"], dtype=np.float32)
    w_out = np.asarray(inputs["w_out"], dtype=np.float32)

    bf = ml_dtypes.bfloat16
    xt_np = np.ascontiguousarray(x.transpose(0, 2, 1)).astype(bf)  # [B, E, S]

    in_maps = []
    for c in range(NCORES):
        rows_q, rows_k, rows_v = [], [], []
        for h in (HPC * c, HPC * c + 1):
            base = h * 3 * D
            rows_q.append(w_qkv[base : base + D])
            rows_k.append(w_qkv[base + D : base + 2 * D])
            rows_v.append(w_qkv[base + 2 * D : base + 3 * D])
        wqk_c = np.concatenate(rows_q + rows_k, axis=0).T  # [E, 4D]
        wv_c = np.concatenate(rows_v, axis=0).T            # [E, 2D]
        wo_c = w_out[:, HPC * c * D : (HPC * c + HPC) * D].T  # [2D, E]
        in_maps.append(
            {
                "xt": xt_np,
                "wqk": np.ascontiguousarray(wqk_c).astype(bf),
                "wv": np.ascontiguousarray(wv_c).astype(bf),
                "wo": np.ascontiguousarray(wo_c).astype(bf),
            }
        )

    if _NC_CACHE is None:
        _NC_CACHE = _build()
    nc = _NC_CACHE

    res = bass_utils.run_bass_kernel_spmd(
        nc, in_maps, core_ids=list(range(NCORES))
    )
    LAST_EXEC_NS = res.exec_time_ns

    y_t = res.results[0]["yt"].astype(np.float32)
    for c in range(1, NCORES):
        y_t += res.results[c]["yt"].astype(np.float32)
    return np.ascontiguousarray(y_t.transpose(0, 2, 1)).astype(np.float32)
